# revision 7
# baseline (speedup 1.0000x reference)
"""L-infinity distance "convolution" kernel for Trainium2 (8 NeuronCores).

Computes out[b, co, h, w] = max_acc |weights[co, acc] - patch[b, h, w, acc]| + bias[co]
where patches are 3x3 replicate-padded windows over x (4, 16, 64, 64),
acc = (c, kh, kw) ordered, accl = 16*9 = 144, cout = 64.

Sharding: 8 cores = 4 batches x 2 row-halves. Each core computes a
[2048 positions, 64 cout] shard. No collectives needed.

v2 design (163 us -> ~42 us):
- All compute operands are fp16 (the correctness gate is rel_err < 2e-2;
  fp16 rounding of patches/weights costs ~1e-3 max rel err). This makes
  every operand of the segmented scan-max op a packed 2-byte SBUF stream,
  which qualifies the DVE's high-performance modes (2 or 4 elements per
  lane-cycle). The custom op ships perf-mode uop table programs
  (uops_2x + fallback slots, perf_max=3 i.e. up to 4X_2PORT), so the
  scan runs at 0.26 ns/elem instead of 1.04: 2.4 us per 128-position
  tile instead of 9.6.
- The op writes the full scan stream [P, cout, 144] (packed innermost, a
  perf-mode requirement) instead of the stride-0 "squash" output; the
  per-cout page maxima live at column 143 of each page. A single Pool
  tensor_tensor add extracts that strided column and adds the bias in one
  go (out fp32).
- No on-chip weight broadcast: the host pre-replicates weights across the
  128 partitions ([128, 9216] fp16) and the DMA streams them in two
  cout-chunks ([8, 56]) so the first compute op starts at ~2.3 us while
  the bulk chunk lands at ~8 us. Patches are laid out partition-major
  ([128, 16*144] fp16, partition p holds positions p, 128+p, ...) so the
  whole patch load is 2 DMAs with 2.3KB contiguous descriptors. Total
  HBM traffic/core ~3.4 MB, ~11 us of DMA fully overlapped with compute.
- Tiles 0..1 consume the weight ladder ([8, 56] couts) as two scan ops;
  tiles 2+ run one full-width 64-cout op (9216 elems) each, minimizing
  the per-instruction SBUF-access init (~60 ns).
"""

import numpy as np

B, C, H, W = 4, 16, 64, 64
K = 3
COUT = 64
ACC = C * K * K  # 144
HOUT, WOUT = 64, 64
NPOS = HOUT * WOUT  # 4096
NCORES = 8
HALVES = 2
POS_PER_CORE = NPOS // HALVES  # 2048
P = 128  # partitions
NTILES = POS_PER_CORE // P  # 16

CFG = {
    "w_chunks": [8, 56],  # cout ladder: small chunk first for early start
    "pt_chunks": [4, 12],  # patch load split (tiles per DMA)
    "stream_bufs": 3,
    "dist_bufs": 4,
}

_TRACE = False

_OP_CACHE = None


def _lower_segscan(spec, ver):
    """Hand-lowered 3-state FSM for a SEGMENTED scan: seed -> steady, with a
    SUB_DIM_DONE step state that re-seeds the scan recurrence on the first
    element of each [P, S, N] page (computing op(init, expr) instead of
    op(carry, expr)). The stock lower() has no per-page reset for regular
    scans; this provides one, giving per-page reductions from one
    instruction. HW-verified bit-exact (fp32); fp16 operands round inputs
    only (max/sub are exact in the scan)."""
    import concourse.dve_spec as ds
    from concourse.dve_spec import Trigger

    n_lanes, n_stages = ds.N_LANES[ver], ds.N_STAGES[ver]
    ds._validate_body(spec, ver)
    spec2 = ds._hoist_stream_invariant_ops(spec)
    scans = ds._collect(spec2.body, ds.Scan)
    latches = ds._collect(spec2.body, ds.Latch)
    assert not latches and spec2.accum is None
    p = ds._build_placement(spec2, scans, n_stages, n_lanes)
    seed_ov, step_ov0 = ds._scan_overrides(scans, p.node_stage)
    assert not step_ov0  # regular scans only (no PageIdx)
    step_ov = {}
    for sc in scans:
        d = p.node_stage[sc]
        step_ov[d] = ds._Stage(sc.op, ds._scan_init(sc), sc.expr)
    body_lvs = ds._body_scan_leaves(spec2)
    consume = (ds.Src0 in body_lvs, ds.Src1 in body_lvs)
    states = [
        ds._State(
            placement=p,
            overrides=seed_ov,
            trigger=ds.COUNT_ONCE,
            repeat=1,
            next=(1, 0, 0),
            write_out=False,
        ),
        ds._State(
            placement=p,
            consume=consume,
            trigger=(Trigger.SRC_TENSOR_DONE, Trigger.SUB_DIM_DONE, Trigger.NONE),
            next=(0, 2, 0),
        ),
        ds._State(
            placement=p,
            consume=consume,
            overrides=step_ov,
            trigger=(Trigger.SRC_TENSOR_DONE, Trigger.SUB_DIM_DONE, Trigger.COUNT),
            next=(0, 2, 1),
            repeat=1,
        ),
    ]
    out = [ds._assemble(s) for s in states]
    for u in out:
        u.validate(ver)
    return out


def _get_op():
    """Register (once) the segmented |a-b| scan-max custom DVE op, with
    perf-mode table slots populated (perf_max=3 -> 2X_1PORT/2X_2PORT/
    4X_2PORT reachable). The perf-mode variants run the same 3-state FSM
    at pair/quad lane granularity; page length 144 is divisible by 4, so
    the per-page running max is preserved at each page's last element."""
    global _OP_CACHE
    if _OP_CACHE is not None:
        return _OP_CACHE
    from concourse.dve_spec import Spec, Src0, Src1, maxx, AluOp, scan
    from concourse.dve_uop import DveOpSpec
    import concourse.dve_ops as dve_ops
    from concourse.dve_ops import DveOp

    def _ref(in0, in1, s0, s1, imm2):
        b = np.maximum.accumulate(
            np.abs(in0.astype(np.float32) - in1.astype(np.float32)), axis=-1
        )
        return b.astype(np.float32)

    spec = Spec(body=scan(AluOp.MAX, maxx(Src0 - Src1, Src1 - Src0)), reference=_ref)
    name = "ABSDIFF_MAX_SEGSCAN"
    if name not in dve_ops._SUB_OPCODE_FOR_NAME:
        row = max(dve_ops._SUB_OPCODE_FOR_NAME.values()) + 1
        assert row < 0x20
        dve_ops._SUB_OPCODE_FOR_NAME[name] = row
    row = dve_ops._SUB_OPCODE_FOR_NAME[name]
    shas = {}
    for ver in ("v3", "v4"):
        uops = _lower_segscan(spec, ver)
        s = DveOpSpec(
            name=name,
            opcode=row,
            uops=uops,
            uops_2x=list(uops),
            perf_max=3,
            rd1_en=True,
        )
        # Pre-populate the compile cache so DveOp.compile() returns the
        # hand-lowered program instead of re-running the stock lower().
        dve_ops._COMPILE_CACHE[(name, ver)] = s
        shas[ver] = s.sha(ver)
    op = DveOp(name, spec, subdim=True, uops_sha=shas, perf_en={"v3": True, "v4": True})
    if all(o.name != name for o in dve_ops.OPS):
        dve_ops.OPS.append(op)
        dve_ops.CUSTOM_DVE_SPECS[name] = spec
    _OP_CACHE = op
    return op


def _build_bass():
    import concourse.bass as bass
    import concourse.bacc as bacc
    import concourse.mybir as mybir
    import concourse.tile as tile
    from concourse.alu_op_type import AluOpType

    op = _get_op()

    nc = bacc.Bacc("TRN2", target_bir_lowering=False, debug=False, num_devices=NCORES)
    patches_d = nc.dram_tensor(
        "patches", [P, NTILES * ACC], mybir.dt.float16, kind="ExternalInput"
    )
    w_d = nc.dram_tensor("w", [P, COUT * ACC], mybir.dt.float16, kind="ExternalInput")
    bias_d = nc.dram_tensor("bias", [P, COUT], mybir.dt.float32, kind="ExternalInput")
    out_d = nc.dram_tensor(
        "out", [POS_PER_CORE, COUT], mybir.dt.float32, kind="ExternalOutput"
    )

    chunk_sizes = CFG["w_chunks"]
    assert sum(chunk_sizes) == COUT
    starts = [sum(chunk_sizes[:i]) for i in range(len(chunk_sizes))]
    nch = len(chunk_sizes)

    custom_insts = []

    with tile.TileContext(nc) as tc:
        with (
            tc.tile_pool(name="consts", bufs=1) as consts,
            tc.tile_pool(name="outp", bufs=CFG["stream_bufs"]) as outp,
        ):
            # DMA order (transfers serialize on the DMA engines): first weight
            # chunk, patches, bulk weights, bias. First scan op starts ~3.4 us;
            # the 8-cout pass keeps the DVE fed until the 56-cout chunk lands.
            wfull = consts.tile([P, COUT * ACC], mybir.dt.float16)
            c0, sz = starts[0] * ACC, chunk_sizes[0] * ACC
            nc.sync.dma_start(out=wfull[:, c0 : c0 + sz], in_=w_d[:, c0 : c0 + sz])
            ptall = consts.tile([P, NTILES * ACC], mybir.dt.float16)
            pt_done = 0
            for ntile in CFG["pt_chunks"]:
                a0, a1 = pt_done * ACC, (pt_done + ntile) * ACC
                nc.sync.dma_start(out=ptall[:, a0:a1], in_=patches_d[:, a0:a1])
                pt_done += ntile
            assert pt_done == NTILES
            for g in range(1, nch):
                c0, sz = starts[g] * ACC, chunk_sizes[g] * ACC
                nc.sync.dma_start(out=wfull[:, c0 : c0 + sz], in_=w_d[:, c0 : c0 + sz])
            bias_rep = consts.tile([P, COUT], mybir.dt.float32)
            nc.sync.dma_start(out=bias_rep[:, :], in_=bias_d[:, :])

            dists = []
            for t in range(NTILES):
                dists.append(
                    outp.tile(
                        [P, COUT],
                        mybir.dt.float32,
                        name=f"dist{t}",
                        tag="dist",
                        bufs=CFG["dist_bufs"],
                    )
                )

            # Chunk-major: pass g processes weight chunk g for every tile, so
            # the DVE always has ready work while later chunks stream in. Each
            # (tile, chunk) scan is followed by a Pool op that extracts the
            # per-cout page maxima (column 143 of each 144-page) and adds the
            # bias, so the big stream scratch recycles immediately.
            for g in range(nch):
                s0, szc = starts[g], chunk_sizes[g]
                w3 = wfull[:, s0 * ACC : (s0 + szc) * ACC].rearrange(
                    "p (c a) -> p c a", a=ACC
                )
                for t in range(NTILES):
                    stream = outp.tile(
                        [P, szc * ACC], mybir.dt.float16, tag=f"stream{g}"
                    )
                    pt = ptall[:, t * ACC : (t + 1) * ACC]
                    o3 = stream[:, :].rearrange("p (c a) -> p c a", a=ACC)
                    ptb = pt.unsqueeze(1).broadcast_to([P, szc, ACC])
                    inst = nc.vector._custom_dve(op, out=o3, in0=w3, in1=ptb)
                    custom_insts.append(inst)
                    sAll = stream[:, 0:szc]
                    col = bass.AP(
                        tensor=sAll.tensor,
                        offset=sAll.offset + (ACC - 1),
                        ap=[sAll.ap[0], [ACC, szc]],
                    )
                    nc.gpsimd.tensor_tensor(
                        out=dists[t][:, s0 : s0 + szc],
                        in0=col,
                        in1=bias_rep[:, s0 : s0 + szc],
                        op=AluOpType.add,
                    )
                    if g == nch - 1:
                        nc.sync.dma_start(
                            out=out_d[t * P : (t + 1) * P, :], in_=dists[t][:, :]
                        )
    # Unlock the DVE high-performance modes on the scan instructions (the
    # table programs for the 2x/2x_2p/4x slots are registered in _get_op).
    for inst in custom_insts:
        inst.ins.perf_max = 3
    nc.compile()
    return nc


def _host_prep(inputs):
    x = np.asarray(inputs["x"], dtype=np.float32)
    weights = np.asarray(inputs["weights"], dtype=np.float32)
    bias = np.asarray(inputs["bias"], dtype=np.float32)
    assert x.shape == (B, C, H, W)
    assert weights.shape == (COUT, ACC)

    x_pad = np.pad(x, ((0, 0), (0, 0), (1, 1), (1, 1)), mode="edge")
    from numpy.lib.stride_tricks import sliding_window_view

    pw = sliding_window_view(x_pad, (K, K), axis=(2, 3))  # (B, C, HOUT, WOUT, K, K)
    patches = (
        np.ascontiguousarray(pw.transpose(0, 2, 3, 1, 4, 5))
        .reshape(B, NPOS, ACC)
        .astype(np.float16)
    )
    wrep = np.ascontiguousarray(
        np.broadcast_to(
            weights.astype(np.float16).reshape(1, COUT * ACC), (P, COUT * ACC)
        )
    )
    brep = np.ascontiguousarray(np.broadcast_to(bias.reshape(1, COUT), (P, COUT)))
    return patches, wrep, brep


_NC_CACHE = None


def _get_nc():
    global _NC_CACHE
    if _NC_CACHE is None:
        _NC_CACHE = _build_bass()
    return _NC_CACHE


def _run(inputs, trace=False):
    from concourse.bass_utils import run_bass_kernel_spmd

    patches, wrep, brep = _host_prep(inputs)
    in_maps = []
    for core in range(NCORES):
        b, half = core // HALVES, core % HALVES
        shard = patches[b, half * POS_PER_CORE : (half + 1) * POS_PER_CORE, :]
        # partition-major: partition p holds positions p, 128+p, ... as
        # NTILES consecutive 144-element patch blocks.
        shard = np.ascontiguousarray(
            shard.reshape(NTILES, P, ACC).transpose(1, 0, 2).reshape(P, NTILES * ACC)
        )
        in_maps.append({"patches": shard, "w": wrep, "bias": brep})

    nc = _get_nc()
    res = run_bass_kernel_spmd(nc, in_maps, core_ids=list(range(NCORES)), trace=trace)

    rows_per_half = POS_PER_CORE // WOUT  # 32
    out = np.empty((B, COUT, HOUT, WOUT), dtype=np.float32)
    for core in range(NCORES):
        b, half = core // HALVES, core % HALVES
        o = res.results[core]["out"]  # [POS_PER_CORE, COUT]
        out[b, :, half * rows_per_half : (half + 1) * rows_per_half, :] = o.T.reshape(
            COUT, rows_per_half, WOUT
        )
    return out, res


def kernel(**inputs) -> np.ndarray:
    out, _ = _run(inputs, trace=_TRACE)
    return out


# revision 8
# speedup vs baseline: 1.0786x; 1.0786x over previous
"""L-infinity distance "convolution" kernel for Trainium2 (8 NeuronCores).

Computes out[b, co, h, w] = max_acc |weights[co, acc] - patch[b, h, w, acc]| + bias[co]
where patches are 3x3 replicate-padded windows over x (4, 16, 64, 64),
acc = (c, kh, kw) ordered, accl = 16*9 = 144, cout = 64.

Sharding: 8 cores = 4 batches x 2 row-halves. Each core computes a
[2048 positions, 64 cout] shard. No collectives needed.

v2 design (163 us -> ~42 us):
- All compute operands are fp16 (the correctness gate is rel_err < 2e-2;
  fp16 rounding of patches/weights costs ~1e-3 max rel err). This makes
  every operand of the segmented scan-max op a packed 2-byte SBUF stream,
  which qualifies the DVE's high-performance modes (2 or 4 elements per
  lane-cycle). The custom op ships perf-mode uop table programs
  (uops_2x + fallback slots, perf_max=3 i.e. up to 4X_2PORT), so the
  scan runs at 0.26 ns/elem instead of 1.04: 2.4 us per 128-position
  tile instead of 9.6.
- The op writes the full scan stream [P, cout, 144] (packed innermost, a
  perf-mode requirement) instead of the stride-0 "squash" output; the
  per-cout page maxima live at column 143 of each page. A single Pool
  tensor_tensor add extracts that strided column and adds the bias in one
  go (out fp32).
- No on-chip weight broadcast: the host pre-replicates weights across the
  128 partitions ([128, 9216] fp16) and the DMA streams them in two
  cout-chunks ([8, 56]) so the first compute op starts at ~2.3 us while
  the bulk chunk lands at ~8 us. Patches are laid out partition-major
  ([128, 16*144] fp16, partition p holds positions p, 128+p, ...) so the
  whole patch load is 2 DMAs with 2.3KB contiguous descriptors. Total
  HBM traffic/core ~3.4 MB, ~11 us of DMA fully overlapped with compute.
- Tiles 0..1 consume the weight ladder ([8, 56] couts) as two scan ops;
  tiles 2+ run one full-width 64-cout op (9216 elems) each, minimizing
  the per-instruction SBUF-access init (~60 ns).
"""

import numpy as np

B, C, H, W = 4, 16, 64, 64
K = 3
COUT = 64
ACC = C * K * K  # 144
HOUT, WOUT = 64, 64
NPOS = HOUT * WOUT  # 4096
NCORES = 8
HALVES = 2
POS_PER_CORE = NPOS // HALVES  # 2048
P = 128  # partitions
NTILES = POS_PER_CORE // P  # 16

CFG = {
    "w_chunks": [8, 56],  # cout ladder: small chunk first for early start
    "pt_chunks": [4, 12],  # patch load split (tiles per DMA)
    "stream_bufs": 3,
    "dist_bufs": 4,
}

_TRACE = False

_OP_CACHE = None


def _lower_segscan(spec, ver):
    """Hand-lowered 3-state FSM for a SEGMENTED scan: seed -> steady, with a
    SUB_DIM_DONE step state that re-seeds the scan recurrence on the first
    element of each [P, S, N] page (computing op(init, expr) instead of
    op(carry, expr)). The stock lower() has no per-page reset for regular
    scans; this provides one, giving per-page reductions from one
    instruction. HW-verified bit-exact (fp32); fp16 operands round inputs
    only (max/sub are exact in the scan)."""
    import concourse.dve_spec as ds
    from concourse.dve_spec import Trigger

    n_lanes, n_stages = ds.N_LANES[ver], ds.N_STAGES[ver]
    ds._validate_body(spec, ver)
    spec2 = ds._hoist_stream_invariant_ops(spec)
    scans = ds._collect(spec2.body, ds.Scan)
    latches = ds._collect(spec2.body, ds.Latch)
    assert not latches and spec2.accum is None
    p = ds._build_placement(spec2, scans, n_stages, n_lanes)
    seed_ov, step_ov0 = ds._scan_overrides(scans, p.node_stage)
    assert not step_ov0  # regular scans only (no PageIdx)
    step_ov = {}
    for sc in scans:
        d = p.node_stage[sc]
        step_ov[d] = ds._Stage(sc.op, ds._scan_init(sc), sc.expr)
    body_lvs = ds._body_scan_leaves(spec2)
    consume = (ds.Src0 in body_lvs, ds.Src1 in body_lvs)
    states = [
        ds._State(
            placement=p,
            overrides=seed_ov,
            trigger=ds.COUNT_ONCE,
            repeat=1,
            next=(1, 0, 0),
            write_out=False,
        ),
        ds._State(
            placement=p,
            consume=consume,
            trigger=(Trigger.SRC_TENSOR_DONE, Trigger.SUB_DIM_DONE, Trigger.NONE),
            next=(0, 2, 0),
        ),
        ds._State(
            placement=p,
            consume=consume,
            overrides=step_ov,
            trigger=(Trigger.SRC_TENSOR_DONE, Trigger.SUB_DIM_DONE, Trigger.COUNT),
            next=(0, 2, 1),
            repeat=1,
        ),
    ]
    out = [ds._assemble(s) for s in states]
    for u in out:
        u.validate(ver)
    return out


def _get_op():
    """Register (once) the segmented |a-b| scan-max custom DVE op, with
    perf-mode table slots populated (perf_max=3 -> 2X_1PORT/2X_2PORT/
    4X_2PORT reachable). The perf-mode variants run the same 3-state FSM
    at pair/quad lane granularity; page length 144 is divisible by 4, so
    the per-page running max is preserved at each page's last element."""
    global _OP_CACHE
    if _OP_CACHE is not None:
        return _OP_CACHE
    from concourse.dve_spec import Spec, Src0, Src1, maxx, AluOp, scan
    from concourse.dve_uop import DveOpSpec
    import concourse.dve_ops as dve_ops
    from concourse.dve_ops import DveOp

    def _ref(in0, in1, s0, s1, imm2):
        b = np.maximum.accumulate(
            np.abs(in0.astype(np.float32) - in1.astype(np.float32)), axis=-1
        )
        return b.astype(np.float32)

    spec = Spec(body=scan(AluOp.MAX, maxx(Src0 - Src1, Src1 - Src0)), reference=_ref)
    name = "ABSDIFF_MAX_SEGSCAN"
    if name not in dve_ops._SUB_OPCODE_FOR_NAME:
        row = max(dve_ops._SUB_OPCODE_FOR_NAME.values()) + 1
        assert row < 0x20
        dve_ops._SUB_OPCODE_FOR_NAME[name] = row
    row = dve_ops._SUB_OPCODE_FOR_NAME[name]
    shas = {}
    for ver in ("v3", "v4"):
        uops = _lower_segscan(spec, ver)
        s = DveOpSpec(
            name=name,
            opcode=row,
            uops=uops,
            uops_2x=list(uops),
            perf_max=3,
            rd1_en=True,
        )
        # Pre-populate the compile cache so DveOp.compile() returns the
        # hand-lowered program instead of re-running the stock lower().
        dve_ops._COMPILE_CACHE[(name, ver)] = s
        shas[ver] = s.sha(ver)
    op = DveOp(name, spec, subdim=True, uops_sha=shas, perf_en={"v3": True, "v4": True})
    if all(o.name != name for o in dve_ops.OPS):
        dve_ops.OPS.append(op)
        dve_ops.CUSTOM_DVE_SPECS[name] = spec
    _OP_CACHE = op
    return op


def _build_bass():
    import concourse.bass as bass
    import concourse.bacc as bacc
    import concourse.mybir as mybir
    import concourse.tile as tile
    from concourse.alu_op_type import AluOpType

    op = _get_op()

    nc = bacc.Bacc("TRN2", target_bir_lowering=False, debug=False, num_devices=NCORES)
    patches_d = nc.dram_tensor(
        "patches", [P, NTILES * ACC], mybir.dt.float16, kind="ExternalInput"
    )
    w_d = nc.dram_tensor("w", [P, COUT * ACC], mybir.dt.float16, kind="ExternalInput")
    bias_d = nc.dram_tensor("bias", [P, COUT], mybir.dt.float32, kind="ExternalInput")
    out_d = nc.dram_tensor(
        "out", [POS_PER_CORE, COUT], mybir.dt.float32, kind="ExternalOutput"
    )

    chunk_sizes = CFG["w_chunks"]
    assert sum(chunk_sizes) == COUT
    starts = [sum(chunk_sizes[:i]) for i in range(len(chunk_sizes))]
    nch = len(chunk_sizes)

    custom_insts = []

    with tile.TileContext(nc) as tc:
        with (
            tc.tile_pool(name="consts", bufs=1) as consts,
            tc.tile_pool(name="outp", bufs=CFG["stream_bufs"]) as outp,
        ):
            # DMA order (transfers serialize on the DMA engines): first weight
            # chunk, patches, bulk weights, bias. First scan op starts ~3.4 us;
            # the 8-cout pass keeps the DVE fed until the 56-cout chunk lands.
            wfull = consts.tile([P, COUT * ACC], mybir.dt.float16)
            c0, sz = starts[0] * ACC, chunk_sizes[0] * ACC
            nc.sync.dma_start(out=wfull[:, c0 : c0 + sz], in_=w_d[:, c0 : c0 + sz])
            ptall = consts.tile([P, NTILES * ACC], mybir.dt.float16)
            pt_done = 0
            for ntile in CFG["pt_chunks"]:
                a0, a1 = pt_done * ACC, (pt_done + ntile) * ACC
                nc.sync.dma_start(out=ptall[:, a0:a1], in_=patches_d[:, a0:a1])
                pt_done += ntile
            assert pt_done == NTILES
            for g in range(1, nch):
                c0, sz = starts[g] * ACC, chunk_sizes[g] * ACC
                nc.sync.dma_start(out=wfull[:, c0 : c0 + sz], in_=w_d[:, c0 : c0 + sz])
            bias_rep = consts.tile([P, COUT], mybir.dt.float32)
            nc.sync.dma_start(out=bias_rep[:, :], in_=bias_d[:, :])

            dists = []
            for t in range(NTILES):
                dists.append(
                    outp.tile(
                        [P, COUT],
                        mybir.dt.float32,
                        name=f"dist{t}",
                        tag=f"dist{t}",
                        bufs=1,
                    )
                )

            # Chunk-major: pass g processes weight chunk g for every tile, so
            # the DVE always has ready work while later chunks stream in. Each
            # (tile, chunk) scan is followed by a Pool op that extracts the
            # per-cout page maxima (column 143 of each 144-page) and adds the
            # bias, so the big stream scratch recycles immediately.
            for g in range(nch):
                s0, szc = starts[g], chunk_sizes[g]
                w3 = wfull[:, s0 * ACC : (s0 + szc) * ACC].rearrange(
                    "p (c a) -> p c a", a=ACC
                )
                for t in range(NTILES):
                    stream = outp.tile(
                        [P, szc * ACC], mybir.dt.float16, tag=f"stream{g}"
                    )
                    pt = ptall[:, t * ACC : (t + 1) * ACC]
                    o3 = stream[:, :].rearrange("p (c a) -> p c a", a=ACC)
                    ptb = pt.unsqueeze(1).broadcast_to([P, szc, ACC])
                    inst = nc.vector._custom_dve(op, out=o3, in0=w3, in1=ptb)
                    custom_insts.append(inst)
                    sAll = stream[:, 0:szc]
                    col = bass.AP(
                        tensor=sAll.tensor,
                        offset=sAll.offset + (ACC - 1),
                        ap=[sAll.ap[0], [ACC, szc]],
                    )
                    nc.gpsimd.tensor_tensor(
                        out=dists[t][:, s0 : s0 + szc],
                        in0=col,
                        in1=bias_rep[:, s0 : s0 + szc],
                        op=AluOpType.add,
                    )
                    if g == nch - 1:
                        nc.sync.dma_start(
                            out=out_d[t * P : (t + 1) * P, :], in_=dists[t][:, :]
                        )
    # Unlock the DVE high-performance modes on the scan instructions (the
    # table programs for the 2x/2x_2p/4x slots are registered in _get_op).
    for inst in custom_insts:
        inst.ins.perf_max = 3
    nc.compile()
    return nc


def _host_prep(inputs):
    x = np.asarray(inputs["x"], dtype=np.float32)
    weights = np.asarray(inputs["weights"], dtype=np.float32)
    bias = np.asarray(inputs["bias"], dtype=np.float32)
    assert x.shape == (B, C, H, W)
    assert weights.shape == (COUT, ACC)

    x_pad = np.pad(x, ((0, 0), (0, 0), (1, 1), (1, 1)), mode="edge")
    from numpy.lib.stride_tricks import sliding_window_view

    pw = sliding_window_view(x_pad, (K, K), axis=(2, 3))  # (B, C, HOUT, WOUT, K, K)
    patches = (
        np.ascontiguousarray(pw.transpose(0, 2, 3, 1, 4, 5))
        .reshape(B, NPOS, ACC)
        .astype(np.float16)
    )
    wrep = np.ascontiguousarray(
        np.broadcast_to(
            weights.astype(np.float16).reshape(1, COUT * ACC), (P, COUT * ACC)
        )
    )
    brep = np.ascontiguousarray(np.broadcast_to(bias.reshape(1, COUT), (P, COUT)))
    return patches, wrep, brep


_NC_CACHE = None


def _get_nc():
    global _NC_CACHE
    if _NC_CACHE is None:
        _NC_CACHE = _build_bass()
    return _NC_CACHE


def _run(inputs, trace=False):
    from concourse.bass_utils import run_bass_kernel_spmd

    patches, wrep, brep = _host_prep(inputs)
    in_maps = []
    for core in range(NCORES):
        b, half = core // HALVES, core % HALVES
        shard = patches[b, half * POS_PER_CORE : (half + 1) * POS_PER_CORE, :]
        # partition-major: partition p holds positions p, 128+p, ... as
        # NTILES consecutive 144-element patch blocks.
        shard = np.ascontiguousarray(
            shard.reshape(NTILES, P, ACC).transpose(1, 0, 2).reshape(P, NTILES * ACC)
        )
        in_maps.append({"patches": shard, "w": wrep, "bias": brep})

    nc = _get_nc()
    res = run_bass_kernel_spmd(nc, in_maps, core_ids=list(range(NCORES)), trace=trace)

    rows_per_half = POS_PER_CORE // WOUT  # 32
    out = np.empty((B, COUT, HOUT, WOUT), dtype=np.float32)
    for core in range(NCORES):
        b, half = core // HALVES, core % HALVES
        o = res.results[core]["out"]  # [POS_PER_CORE, COUT]
        out[b, :, half * rows_per_half : (half + 1) * rows_per_half, :] = o.T.reshape(
            COUT, rows_per_half, WOUT
        )
    return out, res


def kernel(**inputs) -> np.ndarray:
    out, _ = _run(inputs, trace=_TRACE)
    return out


# revision 11
# speedup vs baseline: 1.1660x; 1.0810x over previous
"""L-infinity distance "convolution" kernel for Trainium2 (8 NeuronCores).

Computes out[b, co, h, w] = max_acc |weights[co, acc] - patch[b, h, w, acc]| + bias[co]
where patches are 3x3 replicate-padded windows over x (4, 16, 64, 64),
acc = (c, kh, kw) ordered, accl = 16*9 = 144, cout = 64.

Sharding: 8 cores = 4 batches x 2 row-halves. Each core computes a
[2048 positions, 64 cout] shard. No collectives needed.

v2 design (163 us -> ~42 us):
- All compute operands are fp16 (the correctness gate is rel_err < 2e-2;
  fp16 rounding of patches/weights costs ~1e-3 max rel err). This makes
  every operand of the segmented scan-max op a packed 2-byte SBUF stream,
  which qualifies the DVE's high-performance modes (2 or 4 elements per
  lane-cycle). The custom op ships perf-mode uop table programs
  (uops_2x + fallback slots, perf_max=3 i.e. up to 4X_2PORT), so the
  scan runs at 0.26 ns/elem instead of 1.04: 2.4 us per 128-position
  tile instead of 9.6.
- The op writes the full scan stream [P, cout, 144] (packed innermost, a
  perf-mode requirement) instead of the stride-0 "squash" output; the
  per-cout page maxima live at column 143 of each page. A single Pool
  tensor_tensor add extracts that strided column and adds the bias in one
  go (out fp32).
- No on-chip weight broadcast: the host pre-replicates weights across the
  128 partitions ([128, 9216] fp16) and the DMA streams them in two
  cout-chunks ([8, 56]) so the first compute op starts at ~2.3 us while
  the bulk chunk lands at ~8 us. Patches are laid out partition-major
  ([128, 16*144] fp16, partition p holds positions p, 128+p, ...) so the
  whole patch load is 2 DMAs with 2.3KB contiguous descriptors. Total
  HBM traffic/core ~3.4 MB, ~11 us of DMA fully overlapped with compute.
- Tiles 0..1 consume the weight ladder ([8, 56] couts) as two scan ops;
  tiles 2+ run one full-width 64-cout op (9216 elems) each, minimizing
  the per-instruction SBUF-access init (~60 ns).
"""

import numpy as np

B, C, H, W = 4, 16, 64, 64
K = 3
COUT = 64
ACC = C * K * K  # 144
HOUT, WOUT = 64, 64
NPOS = HOUT * WOUT  # 4096
NCORES = 8
HALVES = 2
POS_PER_CORE = NPOS // HALVES  # 2048
P = 128  # partitions
NTILES = POS_PER_CORE // P  # 16

CFG = {
    "w_chunks": [8, 56],  # cout ladder: small chunk first for early start
    "pt_chunks": [4, 12],  # patch load split (tiles per DMA)
    "stream_bufs": 3,
    "dist_bufs": 4,
}

_TRACE = False

_OP_CACHE = None


def _lower_segscan(spec, ver):
    """Hand-lowered 3-state FSM for a SEGMENTED scan: seed -> steady, with a
    SUB_DIM_DONE step state that re-seeds the scan recurrence on the first
    element of each [P, S, N] page (computing op(init, expr) instead of
    op(carry, expr)). The stock lower() has no per-page reset for regular
    scans; this provides one, giving per-page reductions from one
    instruction. HW-verified bit-exact (fp32); fp16 operands round inputs
    only (max/sub are exact in the scan)."""
    import concourse.dve_spec as ds
    from concourse.dve_spec import Trigger

    n_lanes, n_stages = ds.N_LANES[ver], ds.N_STAGES[ver]
    ds._validate_body(spec, ver)
    spec2 = ds._hoist_stream_invariant_ops(spec)
    scans = ds._collect(spec2.body, ds.Scan)
    latches = ds._collect(spec2.body, ds.Latch)
    assert not latches and spec2.accum is None
    p = ds._build_placement(spec2, scans, n_stages, n_lanes)
    seed_ov, step_ov0 = ds._scan_overrides(scans, p.node_stage)
    assert not step_ov0  # regular scans only (no PageIdx)
    step_ov = {}
    for sc in scans:
        d = p.node_stage[sc]
        step_ov[d] = ds._Stage(sc.op, ds._scan_init(sc), sc.expr)
    body_lvs = ds._body_scan_leaves(spec2)
    consume = (ds.Src0 in body_lvs, ds.Src1 in body_lvs)
    states = [
        ds._State(
            placement=p,
            overrides=seed_ov,
            trigger=ds.COUNT_ONCE,
            repeat=1,
            next=(1, 0, 0),
            write_out=False,
        ),
        ds._State(
            placement=p,
            consume=consume,
            trigger=(Trigger.SRC_TENSOR_DONE, Trigger.SUB_DIM_DONE, Trigger.NONE),
            next=(0, 2, 0),
        ),
        ds._State(
            placement=p,
            consume=consume,
            overrides=step_ov,
            trigger=(Trigger.SRC_TENSOR_DONE, Trigger.SUB_DIM_DONE, Trigger.COUNT),
            next=(0, 2, 1),
            repeat=1,
        ),
    ]
    out = [ds._assemble(s) for s in states]
    for u in out:
        u.validate(ver)
    return out


def _get_op():
    """Register (once) the segmented |a-b| scan-max custom DVE op, with
    perf-mode table slots populated (perf_max=3 -> 2X_1PORT/2X_2PORT/
    4X_2PORT reachable). The perf-mode variants run the same 3-state FSM
    at pair/quad lane granularity; page length 144 is divisible by 4, so
    the per-page running max is preserved at each page's last element."""
    global _OP_CACHE
    if _OP_CACHE is not None:
        return _OP_CACHE
    from concourse.dve_spec import Spec, Src0, Src1, maxx, AluOp, scan
    from concourse.dve_uop import DveOpSpec
    import concourse.dve_ops as dve_ops
    from concourse.dve_ops import DveOp

    def _ref(in0, in1, s0, s1, imm2):
        b = np.maximum.accumulate(
            np.abs(in0.astype(np.float32) - in1.astype(np.float32)), axis=-1
        )
        return b.astype(np.float32)

    spec = Spec(body=scan(AluOp.MAX, maxx(Src0 - Src1, Src1 - Src0)), reference=_ref)
    name = "ABSDIFF_MAX_SEGSCAN"
    if name not in dve_ops._SUB_OPCODE_FOR_NAME:
        row = max(dve_ops._SUB_OPCODE_FOR_NAME.values()) + 1
        assert row < 0x20
        dve_ops._SUB_OPCODE_FOR_NAME[name] = row
    row = dve_ops._SUB_OPCODE_FOR_NAME[name]
    shas = {}
    for ver in ("v3", "v4"):
        uops = _lower_segscan(spec, ver)
        s = DveOpSpec(
            name=name,
            opcode=row,
            uops=uops,
            uops_2x=list(uops),
            perf_max=3,
            rd1_en=True,
        )
        # Pre-populate the compile cache so DveOp.compile() returns the
        # hand-lowered program instead of re-running the stock lower().
        dve_ops._COMPILE_CACHE[(name, ver)] = s
        shas[ver] = s.sha(ver)
    op = DveOp(name, spec, subdim=True, uops_sha=shas, perf_en={"v3": True, "v4": True})
    if all(o.name != name for o in dve_ops.OPS):
        dve_ops.OPS.append(op)
        dve_ops.CUSTOM_DVE_SPECS[name] = spec
    _OP_CACHE = op
    return op


def _build_bass():
    import concourse.bass as bass
    import concourse.bacc as bacc
    import concourse.mybir as mybir
    import concourse.tile as tile
    from concourse.alu_op_type import AluOpType

    op = _get_op()

    nc = bacc.Bacc("TRN2", target_bir_lowering=False, debug=False, num_devices=NCORES)
    patches_d = nc.dram_tensor(
        "patches", [P, NTILES * ACC], mybir.dt.float16, kind="ExternalInput"
    )
    w_d = nc.dram_tensor("w", [P, COUT * ACC], mybir.dt.float16, kind="ExternalInput")
    bias_d = nc.dram_tensor("bias", [P, COUT], mybir.dt.float32, kind="ExternalInput")
    out_d = nc.dram_tensor(
        "out", [POS_PER_CORE, COUT], mybir.dt.float32, kind="ExternalOutput"
    )

    chunk_sizes = CFG["w_chunks"]
    assert sum(chunk_sizes) == COUT
    starts = [sum(chunk_sizes[:i]) for i in range(len(chunk_sizes))]
    nch = len(chunk_sizes)

    custom_insts = []

    with tile.TileContext(nc) as tc:
        with (
            tc.tile_pool(name="consts", bufs=1) as consts,
            tc.tile_pool(name="outp", bufs=CFG["stream_bufs"]) as outp,
        ):
            # DMA order (transfers serialize on the DMA engines): first weight
            # chunk, patches, bulk weights, bias. First scan op starts ~3.4 us;
            # the 8-cout pass keeps the DVE fed until the 56-cout chunk lands.
            wfull = consts.tile([P, COUT * ACC], mybir.dt.float16)
            c0, sz = starts[0] * ACC, chunk_sizes[0] * ACC
            nc.sync.dma_start(out=wfull[:, c0 : c0 + sz], in_=w_d[:, c0 : c0 + sz])
            bias_rep = consts.tile([P, COUT], mybir.dt.float32)
            nc.sync.dma_start(out=bias_rep[:, :], in_=bias_d[:, :])
            ptall = consts.tile([P, NTILES * ACC], mybir.dt.float16)
            pt_done = 0
            for ntile in CFG["pt_chunks"]:
                a0, a1 = pt_done * ACC, (pt_done + ntile) * ACC
                nc.sync.dma_start(out=ptall[:, a0:a1], in_=patches_d[:, a0:a1])
                pt_done += ntile
            assert pt_done == NTILES
            for g in range(1, nch):
                c0, sz = starts[g] * ACC, chunk_sizes[g] * ACC
                nc.sync.dma_start(out=wfull[:, c0 : c0 + sz], in_=w_d[:, c0 : c0 + sz])

            dists = []
            for t in range(NTILES):
                dists.append(
                    outp.tile(
                        [P, COUT],
                        mybir.dt.float32,
                        name=f"dist{t}",
                        tag=f"dist{t}",
                        bufs=1,
                    )
                )

            # Chunk-major: pass g processes weight chunk g for every tile, so
            # the DVE always has ready work while later chunks stream in. Each
            # (tile, chunk) scan is followed by a Pool op that extracts the
            # per-cout page maxima (column 143 of each 144-page) and adds the
            # bias, so the big stream scratch recycles immediately.
            for g in range(nch):
                s0, szc = starts[g], chunk_sizes[g]
                w3 = wfull[:, s0 * ACC : (s0 + szc) * ACC].rearrange(
                    "p (c a) -> p c a", a=ACC
                )
                for t in range(NTILES):
                    stream = outp.tile(
                        [P, szc * ACC],
                        mybir.dt.float16,
                        tag=f"stream{g}",
                        bufs=(6 if szc <= 16 else CFG["stream_bufs"]),
                    )
                    pt = ptall[:, t * ACC : (t + 1) * ACC]
                    o3 = stream[:, :].rearrange("p (c a) -> p c a", a=ACC)
                    ptb = pt.unsqueeze(1).broadcast_to([P, szc, ACC])
                    inst = nc.vector._custom_dve(op, out=o3, in0=w3, in1=ptb)
                    custom_insts.append(inst)
                    sAll = stream[:, 0:szc]
                    col = bass.AP(
                        tensor=sAll.tensor,
                        offset=sAll.offset + (ACC - 1),
                        ap=[sAll.ap[0], [ACC, szc]],
                    )
                    nc.gpsimd.tensor_tensor(
                        out=dists[t][:, s0 : s0 + szc],
                        in0=col,
                        in1=bias_rep[:, s0 : s0 + szc],
                        op=AluOpType.add,
                    )
                    if g == nch - 1:
                        nc.sync.dma_start(
                            out=out_d[t * P : (t + 1) * P, :], in_=dists[t][:, :]
                        )
    # Unlock the DVE high-performance modes on the scan instructions (the
    # table programs for the 2x/2x_2p/4x slots are registered in _get_op).
    for inst in custom_insts:
        inst.ins.perf_max = 3
    nc.compile()
    return nc


def _host_prep(inputs):
    x = np.asarray(inputs["x"], dtype=np.float32)
    weights = np.asarray(inputs["weights"], dtype=np.float32)
    bias = np.asarray(inputs["bias"], dtype=np.float32)
    assert x.shape == (B, C, H, W)
    assert weights.shape == (COUT, ACC)

    x_pad = np.pad(x, ((0, 0), (0, 0), (1, 1), (1, 1)), mode="edge")
    from numpy.lib.stride_tricks import sliding_window_view

    pw = sliding_window_view(x_pad, (K, K), axis=(2, 3))  # (B, C, HOUT, WOUT, K, K)
    patches = (
        np.ascontiguousarray(pw.transpose(0, 2, 3, 1, 4, 5))
        .reshape(B, NPOS, ACC)
        .astype(np.float16)
    )
    wrep = np.ascontiguousarray(
        np.broadcast_to(
            weights.astype(np.float16).reshape(1, COUT * ACC), (P, COUT * ACC)
        )
    )
    brep = np.ascontiguousarray(np.broadcast_to(bias.reshape(1, COUT), (P, COUT)))
    return patches, wrep, brep


_NC_CACHE = None


def _get_nc():
    global _NC_CACHE
    if _NC_CACHE is None:
        _NC_CACHE = _build_bass()
    return _NC_CACHE


def _run(inputs, trace=False):
    from concourse.bass_utils import run_bass_kernel_spmd

    patches, wrep, brep = _host_prep(inputs)
    in_maps = []
    for core in range(NCORES):
        b, half = core // HALVES, core % HALVES
        shard = patches[b, half * POS_PER_CORE : (half + 1) * POS_PER_CORE, :]
        # partition-major: partition p holds positions p, 128+p, ... as
        # NTILES consecutive 144-element patch blocks.
        shard = np.ascontiguousarray(
            shard.reshape(NTILES, P, ACC).transpose(1, 0, 2).reshape(P, NTILES * ACC)
        )
        in_maps.append({"patches": shard, "w": wrep, "bias": brep})

    nc = _get_nc()
    res = run_bass_kernel_spmd(nc, in_maps, core_ids=list(range(NCORES)), trace=trace)

    rows_per_half = POS_PER_CORE // WOUT  # 32
    out = np.empty((B, COUT, HOUT, WOUT), dtype=np.float32)
    for core in range(NCORES):
        b, half = core // HALVES, core % HALVES
        o = res.results[core]["out"]  # [POS_PER_CORE, COUT]
        out[b, :, half * rows_per_half : (half + 1) * rows_per_half, :] = o.T.reshape(
            COUT, rows_per_half, WOUT
        )
    return out, res


def kernel(**inputs) -> np.ndarray:
    out, _ = _run(inputs, trace=_TRACE)
    return out
